# revision 29
# baseline (speedup 1.0000x reference)
"""Bahdanau additive attention, data-parallel over batch on 8 TRN2 NeuronCores.

Math (per batch row b):
    dec_proj = W @ prev[b] + b_W                       # [A]   (host: tiny)
    enc_proj[s] = U @ enc[b,s] + b_U                   # [S, A]
    energy[s] = v . tanh(dec_proj + enc_proj[s])       # [S]
    w = exp(energy);  c[b] = (w @ enc[b]) / sum(w)     # [CTX]

Device strategy (per core, 8 batches), v2 — dual-HBM-pass, zero on-chip
transposes, PE kept warm:
  - enc is staged in BOTH layouts by the host (bf16): natural [s, c] for the
    weighted sum, and transposed [c, s] for the projection.  2 HBM passes of
    1 MiB-contiguous DMAs beat any on-chip transpose path by a wide margin
    (the XBAR route serializes ~2.5 ms on the Sync queue).
  - projection in [a, s] layout: lhsT = U^T chunk [c=128, a=128] (stationary),
    rhs = encT [c=128, s=512] streaming, accumulated over 8 c-chunks in PSUM.
    dec_proj + b_U rides for free as the per-partition bias of the Tanh
    activation (out = tanh(psum + db[a])), output fp16.
  - energy directly as columns: lhsT = tanh-tile [a=128, s=128] (data as
    weights), rhs = v column [a=128, 1]  ->  psum [s=128, 1], 2 a-chunks
    accumulated.  Exp (ScalarE) -> w column [s=128, 1] bf16.
  - weighted sum: lhsT = w column, rhs = natural tile [s=128, c=512]x2 (+ ones
    for the denominator), PSUM-accumulated over the 32 s-tiles of the batch.
  - lag-2 software pipeline (proj(sc) | energy(sc-1) | wsum(sc-2)) so every
    cross-engine dependency has a full stage of slack and the PE never idles
    (HAM stays at K=8/8).
"""

import sys

sys.path.insert(0, "/opt/trn_rl_repo")

import numpy as np
import ml_dtypes

import concourse.bass as bass
from concourse import bacc
import concourse.mybir as mybir
import concourse.tile as tile
from concourse.bass_utils import run_bass_kernel_spmd

B, S, A, DD, CTX = 64, 4096, 256, 1024, 1024
NCORES = 8
BL = B // NCORES   # 8 batches per core
P = 128
KC = CTX // P      # 8 contraction chunks
ACH = A // P       # 2 a-chunks of 128
SC = S // 512      # 8 s-chunks of 512 per batch
ST4 = 512 // P     # 4 s-tiles of 128 per s-chunk
NT = S // P        # 32 s-tiles per batch
NDV = 2            # s-chunks per batch whose weighted sum runs on DVE
BF16 = mybir.dt.bfloat16
FP16 = mybir.dt.float16
F32 = mybir.dt.float32

_CACHE = {}


def _fast_bf16(x: np.ndarray) -> np.ndarray:
    """float32 -> bfloat16 with round-to-nearest-even via integer ops
    (ml_dtypes.astype is ~50x slower on GiB-scale arrays)."""
    u = np.ascontiguousarray(x, dtype=np.float32).view(np.uint32)
    r = ((u + 0x7FFF + ((u >> 16) & 1)) >> 16).astype(np.uint16)
    return r.view(ml_dtypes.bfloat16)


def _build():
    nc = bacc.Bacc()
    enc = nc.declare_dram_parameter("enc", [BL, (SC - NDV) * 512, CTX], BF16,
                                    isOutput=False)
    enct = nc.declare_dram_parameter("enct", [BL, CTX, S], BF16, isOutput=False)
    ut = nc.declare_dram_parameter("ut", [CTX, A], BF16, isOutput=False)
    db = nc.declare_dram_parameter("db", [P, BL * ACH], F32, isOutput=False)
    vv = nc.declare_dram_parameter("vv", [P, ACH], FP16, isOutput=False)
    # PE-path result: c (1024) ++ den (1), un-normalized; host divides.
    out_pe = nc.declare_dram_parameter("out_pe", [BL, CTX + 1], F32,
                                       isOutput=True)
    # DVE-path partial c (c-major) and its softmax-denominator partials.
    out_dv = nc.declare_dram_parameter("out_dv", [BL, KC, P], F32,
                                       isOutput=True)
    den_dv = nc.declare_dram_parameter("den_dv", [BL, max(NDV, 1)], F32,
                                       isOutput=True)

    Tanh = mybir.ActivationFunctionType.Tanh
    Exp = mybir.ActivationFunctionType.Exp
    Copy = mybir.ActivationFunctionType.Copy
    MULT = mybir.AluOpType.mult
    ADD = mybir.AluOpType.add

    with tile.TileContext(nc) as tc:
        with (
            tc.tile_pool(name="const", bufs=1) as const,
            tc.tile_pool(name="data", bufs=3) as data,
            tc.tile_pool(name="natp", bufs=2) as natp,
            tc.tile_pool(name="th", bufs=4) as thp,
            tc.tile_pool(name="wp", bufs=3) as wp,
            tc.tile_pool(name="sm", bufs=1) as sm,
            tc.tile_pool(name="ps", bufs=3, space="PSUM") as ps,
            tc.tile_pool(name="enp", bufs=2, space="PSUM") as enp,
            tc.tile_pool(name="acc", bufs=1, space="PSUM") as accp,
        ):
            # ---- constants, loaded once ----
            ut_sb = const.tile([P, KC, A], BF16)
            nc.sync.dma_start(ut_sb[:], ut.rearrange("(k p) a -> p k a", p=P))
            db_sb = const.tile([P, BL * ACH], F32)
            nc.sync.dma_start(db_sb[:], db[:, :])
            v_sb = const.tile([P, ACH], FP16)
            nc.sync.dma_start(v_sb[:], vv[:, :])
            ones_f32 = const.tile([P, 1], F32)
            nc.vector.memset(ones_f32[:], 1.0)
            ones_row = const.tile([1, P], BF16)
            nc.vector.memset(ones_row[:], 1.0)
            zbias = const.tile([P, 1], F32)
            nc.vector.memset(zbias[:], 0.0)
            # ScalarE clock warmup: observe the DMA and DVE clocks up front so
            # steady-state activations only need their PE (PSUM) wait.
            scr = const.tile([P, BL * ACH], F32)
            nc.scalar.activation(scr[:], db_sb[:], Copy)
            scr2 = const.tile([P, 1], F32)
            nc.scalar.activation(scr2[:], zbias[:], Copy)


            SH = S // 2  # 2048: s-columns per et half-tile

            def load_et(b, h, split=1):
                """DMA one half-batch of transposed enc ([c, s] layout) into a
                fresh [128, KC, 2048] tile.  split>1 cuts each strip into
                s-segments, issued segment-major, so the first s-chunks'
                dependencies clear sooner (batch 0 startup)."""
                et = data.tile([P, KC, SH], BF16, tag="et")
                seg = SH // split
                for q in range(split):
                    for k in range(KC):
                        s0 = h * SH + q * seg
                        nc.sync.dma_start(
                            et[:, k, q * seg:(q + 1) * seg],
                            enct[b, k * P:(k + 1) * P, s0:s0 + seg])
                return et

            def load_nat(b, j0, nj):
                """DMA s-tiles [j0, j0+nj) of natural-layout enc ([s, c]) into
                a fresh [128, nj, CTX] tile."""
                ntile = natp.tile([P, 16, CTX], BF16, tag="nat")
                nc.sync.dma_start(
                    ntile[:, 0:nj, :],
                    enc[b, j0 * P:(j0 + nj) * P, :]
                    .rearrange("(o p) c -> p o c", p=P))
                return ntile

            NPE = SC - NDV           # s-chunks on the PE wsum path
            JPE = NPE * ST4          # s-tiles on the PE wsum path
            NJ1 = min(16, JPE)       # first nat tile s-tiles
            NJ2 = JPE - NJ1          # second nat tile s-tiles

            et_tiles = {(0, 0): load_et(0, 0, split=4), (0, 1): load_et(0, 1)}
            nat_pend = {(0, 0): load_nat(0, 0, NJ1)}
            for b in range(BL):
                c0 = accp.tile([1, 512], F32, tag="c0")
                c1 = accp.tile([1, 512], F32, tag="c1")
                den = accp.tile([1, 1], F32, tag="den")
                cpart = wp.tile([P, KC, max(NDV, 1)], F32, tag="cpart")
                den_d = wp.tile([1, max(NDV, 1)], F32, tag="dend")
                nat_tiles = {h: t for (bb, h), t in nat_pend.items() if bb == b}
                nat_pend = {k: t for k, t in nat_pend.items() if k[0] != b}
                stage = {}  # sc -> (th0, th1) then -> w tile
                for sc in range(SC + 2):
                    # ---- prefetch DMAs (program-order hoisted) ----
                    if sc == 1 and b + 1 < BL:
                        et_tiles[(b + 1, 0)] = load_et(b + 1, 0)
                    if sc == 2 and NJ2 > 0:
                        nat_tiles[1] = load_nat(b, NJ1, NJ2)
                    if sc == 4 and b + 1 < BL:
                        et_tiles[(b + 1, 1)] = load_et(b + 1, 1)
                    if sc == 6 and b + 1 < BL:
                        nat_pend[(b + 1, 0)] = load_nat(b + 1, 0, NJ1)
                    # ---- stage A: projection + tanh for s-chunk sc ----
                    if sc < SC:
                        ths = []
                        eth = et_tiles[(b, sc // 4)]
                        col = (sc % 4) * 512
                        for ach in range(ACH):
                            proj = ps.tile([P, 512], F32, tag="proj")
                            for cch in range(KC):
                                nc.tensor.matmul(
                                    proj[:],
                                    ut_sb[:, cch, ach * P:(ach + 1) * P],
                                    eth[:, cch, col:col + 512],
                                    start=(cch == 0), stop=(cch == KC - 1),
                                )
                            th = thp.tile([P, 512], FP16, tag="th")
                            idx = b * ACH + ach
                            nc.scalar.activation(th[:], proj[:], Tanh,
                                                 bias=db_sb[:, idx:idx + 1])
                            ths.append(th)
                        stage[sc] = ths
                    # ---- stage B: energy + exp for s-chunk sc-1 ----
                    if 1 <= sc <= SC:
                        psc = sc - 1
                        ths = stage[psc]
                        if psc < NPE:
                            # PE wsum path: energy as columns via data-as-
                            # weights matmuls, Exp to w columns.
                            en = enp.tile([P, ST4], F32, tag="en")
                            wt = wp.tile([P, ST4], BF16, tag="w")
                            wacc = wp.tile([P, 1], F32, tag="wacc")
                            for st in range(ST4):
                                for ach in range(ACH):
                                    nc.tensor.matmul(
                                        en[:, st:st + 1],
                                        ths[ach][:, st * P:(st + 1) * P],
                                        v_sb[:, ach:ach + 1],
                                        start=(ach == 0), stop=(ach == ACH - 1),
                                    )
                            # one Exp over all 4 columns; accum_out gives the
                            # per-partition partial softmax denominator free
                            nc.scalar.activation(wt[:, 0:ST4], en[:, 0:ST4],
                                                 Exp, bias=zbias[:],
                                                 accum_out=wacc[:])
                            nc.tensor.matmul(den[:], ones_f32[:], wacc[:],
                                             start=(psc == 0),
                                             stop=(psc == NPE - 1))
                            stage[psc] = wt
                        else:
                            # DVE wsum path: energy as a row (v stationary),
                            # Exp row + its denominator partial via accum_out.
                            qq = psc - NPE
                            erow = enp.tile([1, 512], F32, tag="en")
                            wrow = wp.tile([1, 512], BF16, tag="wrow")
                            for ach in range(ACH):
                                nc.tensor.matmul(
                                    erow[:], v_sb[:, ach:ach + 1],
                                    ths[ach][:, 0:512],
                                    start=(ach == 0), stop=(ach == ACH - 1),
                                )
                            nc.scalar.activation(wrow[:], erow[:], Exp,
                                                 bias=zbias[0:1, :],
                                                 accum_out=den_d[:, qq:qq + 1])
                            stage[psc] = wrow
                    # ---- stage C: weighted sum for s-chunk sc-2 ----
                    if sc >= 2:
                        psc = sc - 2
                        wt = stage.pop(psc)
                        if psc < NPE:
                            for st in range(ST4):
                                j = psc * ST4 + st
                                h, jj = j // 16, j % 16
                                first, last = (j == 0), (j == JPE - 1)
                                nat = nat_tiles[h]
                                nc.tensor.matmul(c0[:], wt[:, st:st + 1],
                                                 nat[:, jj, 0:512],
                                                 start=first, stop=last)
                                nc.tensor.matmul(c1[:], wt[:, st:st + 1],
                                                 nat[:, jj, 512:1024],
                                                 start=first, stop=last)
                        else:
                            # replicate w row across partitions (PE), then
                            # multiply-reduce against the transposed tiles on
                            # the (otherwise idle) Vector engine.
                            qq = psc - NPE
                            wrep_ps = enp.tile([P, 512], F32, tag="en")
                            nc.tensor.matmul(wrep_ps[:], ones_row[:], wt[:],
                                             start=True, stop=True)
                            wrep = wp.tile([P, 512], BF16, tag="wrs")
                            nc.vector.tensor_copy(wrep[:], wrep_ps[:])
                            prod = wp.tile([P, 512], BF16, tag="prod")
                            eth = et_tiles[(b, psc // 4)]
                            col = (psc % 4) * 512
                            for k in range(KC):
                                nc.vector.tensor_mul(
                                    out=prod[:],
                                    in0=eth[:, k, col:col + 512],
                                    in1=wrep[:])
                                nc.vector.tensor_reduce(
                                    cpart[:, k, qq:qq + 1], prod[:],
                                    axis=mybir.AxisListType.X,
                                    op=ADD)

                cout = sm.tile([1, CTX + 1], F32, tag="cout")
                nc.vector.tensor_copy(cout[:, 0:512], c0[:])
                nc.vector.tensor_copy(cout[:, 512:1024], c1[:])
                nc.vector.tensor_copy(cout[:, 1024:1025], den[:])
                nc.sync.dma_start(out_pe[b][None, :], cout[:])
                cdve = sm.tile([P, KC], F32, tag="cdve")
                assert NDV in (1, 2)
                if NDV == 2:
                    nc.vector.tensor_add(out=cdve[:], in0=cpart[:, :, 0],
                                         in1=cpart[:, :, 1])
                else:
                    nc.vector.tensor_copy(cdve[:], cpart[:, :, 0])
                nc.sync.dma_start(out_dv[b].rearrange("k p -> p k"), cdve[:])
                nc.sync.dma_start(den_dv[b][None, :], den_d[:])
                et_tiles.pop((b, 0), None)
                et_tiles.pop((b, 1), None)

    if not nc.is_finalized():
        nc.finalize()
    return nc


def kernel(previous_decoder_hidden_state, encoder_final_hidden_layers,
           W, b_W, U, b_U, v):
    prev = np.asarray(previous_decoder_hidden_state, dtype=np.float32)
    enc = np.asarray(encoder_final_hidden_layers, dtype=np.float32)
    W = np.asarray(W, dtype=np.float32)
    b_W = np.asarray(b_W, dtype=np.float32)
    U = np.asarray(U, dtype=np.float32)
    b_U = np.asarray(b_U, dtype=np.float32)
    v = np.asarray(v, dtype=np.float32)

    if "nc" not in _CACHE:
        _CACHE["nc"] = _build()
    nc = _CACHE["nc"]

    # host-side prep (tiny, except the enc cast which uses a fast bit path)
    db = (prev @ W.T + b_W + b_U).astype(np.float32)            # [B, A]
    db_t = db.reshape(B, ACH, P).transpose(2, 0, 1)             # [P, B, ACH]
    ut = np.ascontiguousarray(U.T).astype(ml_dtypes.bfloat16)   # [CTX, A]
    v2 = np.ascontiguousarray(v.reshape(ACH, P).T).astype(np.float16)  # [P, ACH]
    enc_bf = _fast_bf16(enc)                                    # [B, S, CTX]
    enct_bf = np.ascontiguousarray(enc_bf.transpose(0, 2, 1))   # [B, CTX, S]

    npe_rows = (SC - NDV) * 512  # natural-layout rows the device needs
    in_maps = []
    for i in range(NCORES):
        sl = slice(i * BL, (i + 1) * BL)
        in_maps.append({
            "enc": np.ascontiguousarray(enc_bf[sl, :npe_rows, :]),
            "enct": enct_bf[sl],
            "ut": ut,
            "db": np.ascontiguousarray(db_t[:, sl, :]).reshape(P, BL * ACH),
            "vv": v2,
        })

    res = run_bass_kernel_spmd(nc, in_maps, list(range(NCORES)),
                               **_CACHE.get("run_kwargs", {}))
    _CACHE["last_result"] = res
    outs = []
    for r in res.results:
        pe = np.asarray(r["out_pe"])            # [BL, CTX+1]
        dv = np.asarray(r["out_dv"])            # [BL, KC, P]
        dd = np.asarray(r["den_dv"])            # [BL, NDV]
        c = pe[:, :CTX] + dv.reshape(BL, CTX)
        den = pe[:, CTX] + dd.sum(axis=1)
        outs.append(c / den[:, None])
    return np.concatenate(outs, axis=0).astype(np.float32)
